# revision 37
# baseline (speedup 1.0000x reference)
"""Multi-head causal attention (B=4, S=2048, DM=1024, H=16) on 8 TRN2 cores.

Sharding: core = 2*b + hg  (b = batch 0..3, hg = head-group 0..1, 8 heads each).
Each core computes, for its batch and its 8 heads:
    Q^T = (Wq_hg)^T x^T, K^T likewise   [512, 2048]  (d-major; head pair p's
        even head occupies partitions 0-63 of d-tile p, odd head 64-127)
    V   = x Wv_hg                       [2048, 512]  (s-major, per-head 65-col
                                                      blocks with a ones column
                                                      for the softmax row-sums)
    S^T(kt) = K_kt Q^T per head as K=64 matmuls on disjoint row groups
        (tile_position (0,0)/(64,0) auto-derived) -> masked-exp -> A^T tiles
    O^T_unnorm[65, q] = sum_kt matmul(lhsT=V_aug, rhs=A^T); row 64 = row-sums
        normalized via DVE reciprocal + gpsimd partition_broadcast, with the
        multiply reading the AV PSUM directly
    out_part = O^T-contracted projection with Wo rows of this head group
Host sums the two head-group partials per batch and adds bo.

All matmul inputs bf16 (fp32 PSUM accumulation). Host transposes x and
converts dtypes, so no on-device transposes are needed anywhere.
"""

import sys

for _p in ("/opt/trn_rl_repo",):
    if _p not in sys.path:
        sys.path.insert(0, _p)

import numpy as np
import ml_dtypes

import concourse.bass as bass
import concourse.mybir as mybir
import concourse.tile as tile
from concourse import bacc
from concourse.bass_utils import run_bass_kernel_spmd

B, S, DM, H, DK = 4, 2048, 1024, 16, 64
HPC = 8          # heads per core
DQK = 512        # q/k/v width per core
NPAIR = 4        # head pairs per core (one per 128-row d-tile)
SC = S // 512    # 512-wide q/s chunks
KT = S // 128    # 128-wide key tiles
A_BUFS = 16      # live A^T tiles (pipelined AV keeps few alive)

BF16 = mybir.dt.bfloat16
F32 = mybir.dt.float32
BF = ml_dtypes.bfloat16
EXP = mybir.ActivationFunctionType.Exp
MUL = mybir.AluOpType.mult

LAST_RESULT = None


def _pbcast(row_ap, nparts):
    """Broadcast a [1, N] DRAM AP along the partition axis -> [nparts, N]."""
    return bass.AP(
        tensor=row_ap.tensor,
        offset=row_ap.offset,
        ap=[[0, nparts]] + list(row_ap.ap)[1:],
    )


def _reshape128(row_ap, width):
    """View a [1, 128*width] DRAM AP as [128, width]."""
    return bass.AP(
        tensor=row_ap.tensor,
        offset=row_ap.offset,
        ap=[[width, 128], [1, width]],
    )


def _pbcast2(mask_sb):
    """mask [128, 128] -> [128, 2, 128] with broadcast middle dim."""
    m = mask_sb[:, :]
    ap = list(m.ap)
    return bass.AP(tensor=m.tensor, offset=m.offset, ap=[ap[0], [0, 2], ap[1]])


def _emit(nc, nkt):
    """Emit the whole per-core kernel. nkt = contraction tiles (8, or 9 when
    biases are folded in via an augmented ones-row in xT)."""
    xT = nc.dram_tensor("xT", [nkt * 128, S], BF16, kind="ExternalInput").ap()
    wq = nc.dram_tensor("wq", [nkt * 128, DQK], BF16, kind="ExternalInput").ap()
    wk = nc.dram_tensor("wk", [nkt * 128, DQK], BF16, kind="ExternalInput").ap()
    wv = nc.dram_tensor("wv", [nkt * 128, DQK], BF16, kind="ExternalInput").ap()
    wo = nc.dram_tensor("wo", [DQK, DM], BF16, kind="ExternalInput").ap()
    msk = nc.dram_tensor("mask", [128, 128], BF16, kind="ExternalInput").ap()
    out = nc.dram_tensor("out", [S, DM], F32, kind="ExternalOutput").ap()

    xT3 = xT.rearrange("(t p) s -> p t s", p=128)
    wq3 = wq.rearrange("(t p) d -> p t d", p=128)
    wk3 = wk.rearrange("(t p) d -> p t d", p=128)
    wv3 = wv.rearrange("(t p) d -> p t d", p=128)
    wo3 = wo.rearrange("(t p) m -> p t m", p=128)

    with tile.TileContext(nc) as tc:
        with (
            tc.tile_pool(name="big", bufs=1) as big,
            tc.tile_pool(name="wqkv", bufs=1) as wp,
            tc.tile_pool(name="xc", bufs=2) as xcp,
            tc.tile_pool(name="ap", bufs=A_BUFS) as apool,
            tc.tile_pool(name="rcp", bufs=3) as rcpp,
            tc.tile_pool(name="bc", bufs=4) as bcp,
            tc.tile_pool(name="ob", bufs=2) as obp,
            tc.tile_pool(name="pp", bufs=8) as ppp,
            tc.tile_pool(name="mm", bufs=2, space="PSUM") as mmp,
            tc.tile_pool(name="sc", bufs=2, space="PSUM") as scp,
            tc.tile_pool(name="otp", bufs=2, space="PSUM") as otpp,
            tc.tile_pool(name="rb", bufs=8, space="DRAM") as rbp,
        ):
            # scores matmuls contract over the full 128 partitions with the
            # other pair-half zeroed (row-tiled K=64 matmuls after a full-
            # array matmul hard-fault the device on this toolchain)
            qt_e = big.tile([128, NPAIR, S], BF16, tag="qte")
            qt_o = big.tile([128, NPAIR, S], BF16, tag="qto")
            kt_ = big.tile([128, NPAIR, S], BF16, tag="kt")
            v = big.tile([128, KT, HPC * 65], BF16, tag="v")
            ot = big.tile([128, NPAIR, S], BF16, tag="ot")
            wo_sb = big.tile([128, NPAIR, DM], BF16, tag="wo")
            mask_sb = big.tile([128, 128], BF16, tag="mask")

            wq_sb = wp.tile([128, nkt, DQK], BF16, tag="wq")
            wk_sb = wp.tile([128, nkt, DQK], BF16, tag="wk")
            wv_sb = wp.tile([128, nkt, DQK], BF16, tag="wv")

            # prologue DMAs, halved so the first Q matmuls can start early
            xc0 = xcp.tile([128, nkt, 512], BF16, tag="xc")
            hk = nkt // 2
            nc.sync.dma_start(out=wq_sb[:, 0:hk, :], in_=wq3[:, 0:hk, :])
            nc.sync.dma_start(out=xc0[:, 0:hk, :], in_=xT3[:, 0:hk, 0:512])
            nc.sync.dma_start(out=wq_sb[:, hk:nkt, :], in_=wq3[:, hk:nkt, :])
            nc.sync.dma_start(out=xc0[:, hk:nkt, :], in_=xT3[:, hk:nkt, 0:512])
            nc.sync.dma_start(out=wk_sb[:, 0:hk, :], in_=wk3[:, 0:hk, :])
            nc.sync.dma_start(out=wk_sb[:, hk:nkt, :], in_=wk3[:, hk:nkt, :])
            nc.sync.dma_start(out=mask_sb, in_=msk)
            nc.sync.dma_start(out=wv_sb, in_=wv3)
            nc.sync.dma_start(out=wo_sb, in_=wo3)

            # p-state warmup first (small memset, then matmuls run while
            # the prologue DMAs land so real matmuls start at full clock)
            warm = apool.tile([128, 2, 512], BF16, tag="a")
            nc.vector.memset(warm, 0.0)
            wps = mmp.tile([128, 512], F32, tag="mm")
            NWARM = 12
            for i in range(NWARM):
                nc.tensor.matmul(
                    out=wps,
                    lhsT=warm[:, 0, 0:128],
                    rhs=warm[:, 1, :],
                    start=(i == 0),
                    stop=(i == NWARM - 1),
                )

            # ones columns of the V blocks (col 64 of each 65-wide block)
            v4 = v.rearrange("p t (h c) -> p t h c", c=65)
            nc.vector.memset(v4[:, :, :, 64:65], 1.0)

            def qkv_steps(st, xc=None):
                """Generator: one yield per matmul group (12 total)."""
                if xc is None:
                    xc = xcp.tile([128, nkt, 512], BF16, tag="xc")
                    nc.sync.dma_start(
                        out=xc, in_=xT3[:, :, st * 512 : (st + 1) * 512]
                    )
                ss = slice(st * 512, (st + 1) * 512)
                # zero this stile's columns of the unused pair-halves (kept
                # off the hot DVE path by chunking instead of one big memset)
                nc.vector.memset(qt_e[64:128, :, ss], 0.0)
                nc.vector.memset(qt_o[0:64, :, ss], 0.0)
                for w_sb, dst in ((wq_sb, None), (wk_sb, kt_)):
                    for dt_i in range(NPAIR):
                        ps = mmp.tile([128, 512], F32, tag="mm")
                        for k in range(nkt):
                            nc.tensor.matmul(
                                out=ps,
                                lhsT=w_sb[:, k, dt_i * 128 : (dt_i + 1) * 128],
                                rhs=xc[:, k, :],
                                start=(k == 0),
                                stop=(k == nkt - 1),
                            )
                        if dst is None:
                            nc.vector.tensor_copy(
                                out=qt_e[0:64, dt_i, ss], in_=ps[0:64, :]
                            )
                            nc.vector.tensor_copy(
                                out=qt_o[64:128, dt_i, ss], in_=ps[64:128, :]
                            )
                        else:
                            nc.vector.tensor_copy(out=dst[:, dt_i, ss], in_=ps)
                        yield
                for ssub in range(4):
                    ps = mmp.tile([128, 512], F32, tag="mm")
                    for k in range(nkt):
                        nc.tensor.matmul(
                            out=ps,
                            lhsT=xc[:, k, ssub * 128 : (ssub + 1) * 128],
                            rhs=wv_sb[:, k, :],
                            start=(k == 0),
                            stop=(k == nkt - 1),
                        )
                    kti = st * 4 + ssub
                    ps4 = ps.rearrange("p (h d) -> p h d", d=DK)
                    vv = v4[:, kti, :, :]
                    nc.vector.tensor_copy(out=vv[:, :, 0:DK], in_=ps4)
                    yield

            def attn(pr, c, filler):
                """Attention for head pair pr, q chunk c. After each kti's
                scores+exp, advances the filler iterator once to slot an
                independent matmul group into the PE queue (covers the
                exp-paced stretches)."""
                a_tiles = {}
                opses = {}
                nkts = 4 * c + 4

                def av_mms(k2, stop):
                    qoff2 = 128 * (k2 % 4) if k2 // 4 == c else 0
                    for hh in (1, 0):
                        if k2 == 0:
                            opses[hh] = otpp.tile([128, 512], F32, tag="otp", name=f"ops{pr}{c}{hh}")
                        nc.tensor.matmul(
                            out=opses[hh][0:65, qoff2:512],
                            lhsT=v[:, k2, (pr * 2 + hh) * 65 : (pr * 2 + hh) * 65 + 65],
                            rhs=a_tiles[k2][:, hh, qoff2:512],
                            start=(k2 == 0),
                            stop=stop,
                        )

                for kti in range(nkts):
                    # software-pipelined AV, lag 3, emitted before the scores
                    # so it fills the scores' PSUM-recycle wait
                    if kti >= 3:
                        av_mms(kti - 3, stop=False)
                    diag = kti // 4 == c
                    qoff = 128 * (kti % 4) if diag else 0
                    ps = scp.tile([128, 2, 512], F32, tag="sc")
                    for hh, qsrc in ((0, qt_e), (1, qt_o)):
                        nc.tensor.matmul(
                            out=ps[:, hh, qoff:512],
                            lhsT=kt_[:, pr, kti * 128 : (kti + 1) * 128],
                            rhs=qsrc[:, pr, c * 512 + qoff : (c + 1) * 512],
                            start=True,
                            stop=True,
                        )
                    at = apool.tile([128, 2, 512], BF16, tag="a")
                    nc.scalar.activation(
                        out=at[:, :, qoff:512],
                        in_=ps[:, :, qoff:512],
                        func=EXP,
                        scale=0.125,
                    )
                    if diag:
                        dg = at[:, :, qoff : qoff + 128]
                        nc.vector.tensor_tensor(
                            out=dg, in0=dg, in1=_pbcast2(mask_sb), op=MUL
                        )
                    a_tiles[kti] = at
                    next(filler, None)
                for k2 in range(max(0, nkts - 3), nkts):
                    av_mms(k2, stop=(k2 == nkts - 1))
                csl = slice(c * 512, (c + 1) * 512)
                # phase A for both heads first (copies, reciprocals, DMA
                # issues), then the multiplies: keeps hh0's copy/recip from
                # queuing behind hh1's broadcast-waiting multiply on the
                # in-order DVE queue
                rss, bcs = {}, {}
                for hh in (1, 0):
                    ops = opses[hh]
                    rs = rcpp.tile([128, 512], F32, tag="rs")
                    nc.vector.tensor_copy(out=rs[0:65, :], in_=ops[0:65, :])
                    rr = rcpp.tile([65, 512], F32, tag="rr")
                    nc.vector.reciprocal(out=rr[64:65, :], in_=rs[64:65, :])
                    rd = rbp.tile([1, 512], F32, tag="rb")
                    nc.sync.dma_start(out=rd, in_=rr[64:65, :])
                    bc = bcp.tile([128, 512], F32, tag="bc")
                    nc.sync.dma_start(out=bc[0:64, :], in_=_pbcast(rd, 64))
                    rss[hh], bcs[hh] = rs, bc
                for hh in (1, 0):  # hh=0 last: ends with a DVE write, no DMA
                    rs, bc = rss[hh], bcs[hh]
                    if hh == 0:
                        nc.vector.tensor_tensor(
                            out=ot[0:64, pr, csl],
                            in0=rs[0:64, :],
                            in1=bc[0:64, :],
                            op=MUL,
                        )
                    else:
                        tmp = bcp.tile([64, 512], BF16, tag="tmp")
                        nc.vector.tensor_tensor(
                            out=tmp, in0=rs[0:64, :], in1=bc[0:64, :], op=MUL
                        )
                        nc.sync.dma_start(out=ot[64:128, pr, csl], in_=tmp)

            def proj_steps(chunk, last_pair=None):
                """Generator: one yield per srow group. When last_pair is
                given, that pair's matmul goes last in each accumulation so
                the other three overlap its still-running softmax dance."""
                dts = list(range(NPAIR))
                if last_pair is not None:
                    dts.remove(last_pair)
                    dts.append(last_pair)
                for ssub in range(4):
                    srow = chunk * 4 + ssub
                    ob = obp.tile([128, 2, 512], F32, tag="ob")
                    for mc in range(2):
                        ps = mmp.tile([128, 512], F32, tag="mm")
                        for j, dt_i in enumerate(dts):
                            nc.tensor.matmul(
                                out=ps,
                                lhsT=ot[:, dt_i, srow * 128 : (srow + 1) * 128],
                                rhs=wo_sb[:, dt_i, mc * 512 : (mc + 1) * 512],
                                start=(j == 0),
                                stop=(j == NPAIR - 1),
                            )
                        nc.vector.tensor_copy(out=ob[:, mc, :], in_=ps)
                        nc.sync.dma_start(
                            out=out[
                                srow * 128 : (srow + 1) * 128,
                                mc * 512 : (mc + 1) * 512,
                            ],
                            in_=ob[:, mc, :],
                        )
                    yield

            def proj(chunk, last_pair=None):
                for _ in proj_steps(chunk, last_pair):
                    pass

            pp_tiles = {}

            def proj_last_p1(last_pair):
                """First 3 pairs of the final chunk's projection, closed
                groups copied to bf16 partials so all of it can run during
                the final pair's attention."""
                chunk = SC - 1
                dts = [d for d in range(NPAIR) if d != last_pair]
                for ssub in range(4):
                    srow = chunk * 4 + ssub
                    for mc in range(2):
                        ps = mmp.tile([128, 512], F32, tag="mm")
                        for j, dt_i in enumerate(dts):
                            nc.tensor.matmul(
                                out=ps,
                                lhsT=ot[:, dt_i, srow * 128 : (srow + 1) * 128],
                                rhs=wo_sb[:, dt_i, mc * 512 : (mc + 1) * 512],
                                start=(j == 0),
                                stop=(j == len(dts) - 1),
                            )
                        pp = ppp.tile(
                            [128, 512], BF16, tag="pp", name=f"pp{ssub}{mc}"
                        )
                        nc.vector.tensor_copy(out=pp, in_=ps)
                        pp_tiles[(ssub, mc)] = pp
                        yield

            def proj_last_p2(last_pair):
                chunk = SC - 1
                for ssub in range(4):
                    srow = chunk * 4 + ssub
                    ob = obp.tile([128, 2, 512], F32, tag="ob")
                    for mc in range(2):
                        ps = mmp.tile([128, 512], F32, tag="mm")
                        nc.tensor.matmul(
                            out=ps,
                            lhsT=ot[:, last_pair, srow * 128 : (srow + 1) * 128],
                            rhs=wo_sb[:, last_pair, mc * 512 : (mc + 1) * 512],
                            start=True,
                            stop=True,
                        )
                        t2 = bcp.tile([128, 512], F32, tag="bc")
                        nc.scalar.copy(out=t2, in_=ps)
                        nc.vector.tensor_tensor(
                            out=ob[:, mc, :],
                            in0=t2,
                            in1=pp_tiles[(ssub, mc)],
                            op=mybir.AluOpType.add,
                        )
                        nc.sync.dma_start(
                            out=out[
                                srow * 128 : (srow + 1) * 128,
                                mc * 512 : (mc + 1) * 512,
                            ],
                            in_=ob[:, mc, :],
                        )

            def _seq(*gens):
                for g in gens:
                    yield from g

            def _skips(n):
                for _ in range(n):
                    yield

            empty = iter(())
            for st in range(SC):
                for _ in qkv_steps(st, xc=xc0 if st == 0 else None):
                    pass
                last = st == SC - 1
                if st >= 1 and not last:
                    proj(st - 1)
                pairs = (3, 0, 1, 2) if last else (0, 1, 2, 3)
                # the last stile has no next-stile qkv to fill the exp-paced
                # stretches, so feed proj(SC-2) groups in as filler instead
                filler = proj_steps(SC - 2) if last else empty
                for pr in pairs:
                    attn(pr, st, filler)
                for _ in filler:
                    pass
            # final projection in two passes: the three finished pairs run
            # inside the last softmax-dance latency (keeps PE busy and warm),
            # the last pair joins via a single matmul + SBUF add afterwards
            for _ in proj_last_p1(pairs[-1]):
                pass
            # keep the PE clock warm across the final dance latency so the
            # last-pair matmuls run at full rate
            wps2 = mmp.tile([128, 512], F32, tag="mm")
            for i in range(10):
                nc.tensor.matmul(
                    out=wps2,
                    lhsT=warm[:, 0, 0:128],
                    rhs=warm[:, 1, :],
                    start=(i == 0),
                    stop=(i == 9),
                )
            proj_last_p2(pairs[-1])
    return nc


_NC_CACHE = {}


def _get_nc(nkt):
    if nkt not in _NC_CACHE:
        nc = bacc.Bacc(
            "TRN2",
            target_bir_lowering=False,
            debug=False,
            enable_asserts=False,
            num_devices=8,
        )
        _emit(nc, nkt)
        nc.compile()
        _NC_CACHE[nkt] = nc
    return _NC_CACHE[nkt]


def kernel(**inputs):
    x = np.asarray(inputs["x"], dtype=np.float32)
    mask = np.asarray(inputs["mask"]).reshape(S, S)
    Wq = np.asarray(inputs["Wq"], dtype=np.float32)
    bq = np.asarray(inputs["bq"], dtype=np.float32)
    Wk = np.asarray(inputs["Wk"], dtype=np.float32)
    bk = np.asarray(inputs["bk"], dtype=np.float32)
    Wv = np.asarray(inputs["Wv"], dtype=np.float32)
    bv = np.asarray(inputs["bv"], dtype=np.float32)
    Wo = np.asarray(inputs["Wo"], dtype=np.float32)
    bo = np.asarray(inputs["bo"], dtype=np.float32)

    assert np.array_equal(
        mask, np.tril(np.ones((S, S), dtype=bool))
    ), "kernel specialized for the causal (tril) mask"

    bias_zero = not (bq.any() or bk.any() or bv.any())
    nkt = 8 if bias_zero else 9
    nc = _get_nc(nkt)

    # local diag-block mask in (k, q) layout: valid when q >= k
    mtile = np.triu(np.ones((128, 128), dtype=np.float32)).astype(BF)

    def aug(w, b):
        if bias_zero:
            return w.astype(BF)
        pad = np.zeros((128, w.shape[1]), dtype=np.float32)
        pad[0] = b
        return np.vstack([w, pad]).astype(BF)

    in_maps = []
    for core in range(8):
        b, hg = divmod(core, 2)
        cols = slice(hg * DQK, (hg + 1) * DQK)
        xT = x[b].T
        if not bias_zero:
            pad = np.zeros((128, S), dtype=np.float32)
            pad[0] = 1.0
            xT = np.vstack([xT, pad])
        in_maps.append(
            {
                "xT": np.ascontiguousarray(xT).astype(BF),
                "wq": aug(Wq[:, cols], bq[cols]),
                "wk": aug(Wk[:, cols], bk[cols]),
                "wv": aug(Wv[:, cols], bv[cols]),
                "wo": np.ascontiguousarray(Wo[cols, :]).astype(BF),
                "mask": mtile,
            }
        )

    res = run_bass_kernel_spmd(nc, in_maps, core_ids=list(range(8)))
    global LAST_RESULT
    LAST_RESULT = res
    parts = [r["out"] for r in res.results]
    out = np.stack(
        [parts[2 * b_] + parts[2 * b_ + 1] for b_ in range(B)]
    ) + bo.astype(np.float32)
    return out.astype(np.float32)


# revision 41
# speedup vs baseline: 1.0104x; 1.0104x over previous
"""Multi-head causal attention (B=4, S=2048, DM=1024, H=16) on 8 TRN2 cores.

Sharding: core = 2*b + hg  (b = batch 0..3, hg = head-group 0..1, 8 heads each).
Each core computes, for its batch and its 8 heads:
    Q^T = (Wq_hg)^T x^T, K^T likewise   [512, 2048]  (d-major; head pair p's
        even head occupies partitions 0-63 of d-tile p, odd head 64-127)
    V   = x Wv_hg                       [2048, 512]  (s-major, per-head 65-col
                                                      blocks with a ones column
                                                      for the softmax row-sums)
    S^T(kt) = K_kt Q^T per head as K=64 matmuls on disjoint row groups
        (tile_position (0,0)/(64,0) auto-derived) -> masked-exp -> A^T tiles
    O^T_unnorm[65, q] = sum_kt matmul(lhsT=V_aug, rhs=A^T); row 64 = row-sums
        normalized via DVE reciprocal + gpsimd partition_broadcast, with the
        multiply reading the AV PSUM directly
    out_part = O^T-contracted projection with Wo rows of this head group
Host sums the two head-group partials per batch and adds bo.

All matmul inputs bf16 (fp32 PSUM accumulation). Host transposes x and
converts dtypes, so no on-device transposes are needed anywhere.
"""

import sys

for _p in ("/opt/trn_rl_repo",):
    if _p not in sys.path:
        sys.path.insert(0, _p)

import numpy as np
import ml_dtypes

import concourse.bass as bass
import concourse.mybir as mybir
import concourse.tile as tile
from concourse import bacc
from concourse.bass_utils import run_bass_kernel_spmd

B, S, DM, H, DK = 4, 2048, 1024, 16, 64
HPC = 8          # heads per core
DQK = 512        # q/k/v width per core
NPAIR = 4        # head pairs per core (one per 128-row d-tile)
SC = S // 512    # 512-wide q/s chunks
KT = S // 128    # 128-wide key tiles
A_BUFS = 16      # live A^T tiles (pipelined AV keeps few alive)

BF16 = mybir.dt.bfloat16
F32 = mybir.dt.float32
BF = ml_dtypes.bfloat16
EXP = mybir.ActivationFunctionType.Exp
MUL = mybir.AluOpType.mult

LAST_RESULT = None


def _pbcast(row_ap, nparts):
    """Broadcast a [1, N] DRAM AP along the partition axis -> [nparts, N]."""
    return bass.AP(
        tensor=row_ap.tensor,
        offset=row_ap.offset,
        ap=[[0, nparts]] + list(row_ap.ap)[1:],
    )


def _reshape128(row_ap, width):
    """View a [1, 128*width] DRAM AP as [128, width]."""
    return bass.AP(
        tensor=row_ap.tensor,
        offset=row_ap.offset,
        ap=[[width, 128], [1, width]],
    )


def _pbcast2(mask_sb):
    """mask [128, 128] -> [128, 2, 128] with broadcast middle dim."""
    m = mask_sb[:, :]
    ap = list(m.ap)
    return bass.AP(tensor=m.tensor, offset=m.offset, ap=[ap[0], [0, 2], ap[1]])


def _emit(nc, nkt):
    """Emit the whole per-core kernel. nkt = contraction tiles (8, or 9 when
    biases are folded in via an augmented ones-row in xT)."""
    xT = nc.dram_tensor("xT", [nkt * 128, S], BF16, kind="ExternalInput").ap()
    wq = nc.dram_tensor("wq", [nkt * 128, DQK], BF16, kind="ExternalInput").ap()
    wk = nc.dram_tensor("wk", [nkt * 128, DQK], BF16, kind="ExternalInput").ap()
    wv = nc.dram_tensor("wv", [nkt * 128, DQK], BF16, kind="ExternalInput").ap()
    wo = nc.dram_tensor("wo", [DQK, DM], BF16, kind="ExternalInput").ap()
    msk = nc.dram_tensor("mask", [128, 128], BF16, kind="ExternalInput").ap()
    out = nc.dram_tensor("out", [S, DM], F32, kind="ExternalOutput").ap()

    xT3 = xT.rearrange("(t p) s -> p t s", p=128)
    wq3 = wq.rearrange("(t p) d -> p t d", p=128)
    wk3 = wk.rearrange("(t p) d -> p t d", p=128)
    wv3 = wv.rearrange("(t p) d -> p t d", p=128)
    wo3 = wo.rearrange("(t p) m -> p t m", p=128)

    with tile.TileContext(nc) as tc:
        with (
            tc.tile_pool(name="big", bufs=1) as big,
            tc.tile_pool(name="wqkv", bufs=1) as wp,
            tc.tile_pool(name="xc", bufs=2) as xcp,
            tc.tile_pool(name="ap", bufs=A_BUFS) as apool,
            tc.tile_pool(name="rcp", bufs=3) as rcpp,
            tc.tile_pool(name="bc", bufs=4) as bcp,
            tc.tile_pool(name="ob", bufs=2) as obp,
            tc.tile_pool(name="pp", bufs=8) as ppp,
            tc.tile_pool(name="mm", bufs=2, space="PSUM") as mmp,
            tc.tile_pool(name="sc", bufs=2, space="PSUM") as scp,
            tc.tile_pool(name="otp", bufs=2, space="PSUM") as otpp,
            tc.tile_pool(name="rb", bufs=8, space="DRAM") as rbp,
        ):
            # scores matmuls contract over the full 128 partitions with the
            # other pair-half zeroed (row-tiled K=64 matmuls after a full-
            # array matmul hard-fault the device on this toolchain)
            qt_e = big.tile([128, NPAIR, S], BF16, tag="qte")
            qt_o = big.tile([128, NPAIR, S], BF16, tag="qto")
            kt_ = big.tile([128, NPAIR, S], BF16, tag="kt")
            v = big.tile([128, KT, HPC * 65], BF16, tag="v")
            ot = big.tile([128, NPAIR, S], BF16, tag="ot")
            wo_sb = big.tile([128, NPAIR, DM], BF16, tag="wo")
            mask_sb = big.tile([128, 128], BF16, tag="mask")

            wq_sb = wp.tile([128, nkt, DQK], BF16, tag="wq")
            wk_sb = wp.tile([128, nkt, DQK], BF16, tag="wk")
            wv_sb = wp.tile([128, nkt, DQK], BF16, tag="wv")

            # prologue DMAs, halved so the first Q matmuls can start early
            xc0 = xcp.tile([128, nkt, 512], BF16, tag="xc")
            hk = nkt // 2
            nc.sync.dma_start(out=wq_sb[:, 0:hk, :], in_=wq3[:, 0:hk, :])
            nc.sync.dma_start(out=xc0[:, 0:hk, :], in_=xT3[:, 0:hk, 0:512])
            nc.sync.dma_start(out=wq_sb[:, hk:nkt, :], in_=wq3[:, hk:nkt, :])
            nc.sync.dma_start(out=xc0[:, hk:nkt, :], in_=xT3[:, hk:nkt, 0:512])
            nc.sync.dma_start(out=wk_sb[:, 0:hk, :], in_=wk3[:, 0:hk, :])
            nc.sync.dma_start(out=wk_sb[:, hk:nkt, :], in_=wk3[:, hk:nkt, :])
            nc.sync.dma_start(out=mask_sb, in_=msk)
            nc.sync.dma_start(out=wv_sb, in_=wv3)
            nc.sync.dma_start(out=wo_sb, in_=wo3)

            # p-state warmup first (small memset, then matmuls run while
            # the prologue DMAs land so real matmuls start at full clock)
            warm = apool.tile([128, 2, 512], BF16, tag="a")
            nc.vector.memset(warm, 0.0)
            wps = mmp.tile([128, 512], F32, tag="mm")
            NWARM = 12
            for i in range(NWARM):
                nc.tensor.matmul(
                    out=wps,
                    lhsT=warm[:, 0, 0:128],
                    rhs=warm[:, 1, :],
                    start=(i == 0),
                    stop=(i == NWARM - 1),
                )

            # ones columns of the V blocks (col 64 of each 65-wide block)
            v4 = v.rearrange("p t (h c) -> p t h c", c=65)
            nc.vector.memset(v4[:, :, :, 64:65], 1.0)

            def qkv_steps(st, xc=None):
                """Generator: one yield per matmul group (12 total)."""
                if xc is None:
                    xc = xcp.tile([128, nkt, 512], BF16, tag="xc")
                    nc.sync.dma_start(
                        out=xc, in_=xT3[:, :, st * 512 : (st + 1) * 512]
                    )
                ss = slice(st * 512, (st + 1) * 512)
                # zero this stile's columns of the unused pair-halves (kept
                # off the hot DVE path by chunking instead of one big memset)
                nc.vector.memset(qt_e[64:128, :, ss], 0.0)
                nc.vector.memset(qt_o[0:64, :, ss], 0.0)
                # interleave Q and K groups so the K matmuls (whose weights
                # arrive later) don't all queue at the end of the stile
                for w_sb, dst, dt_i in (
                    (wq_sb, None, 0),
                    (wq_sb, None, 1),
                    (wk_sb, kt_, 0),
                    (wq_sb, None, 2),
                    (wk_sb, kt_, 1),
                    (wq_sb, None, 3),
                    (wk_sb, kt_, 2),
                    (wk_sb, kt_, 3),
                ):
                    ps = mmp.tile([128, 512], F32, tag="mm")
                    for k in range(nkt):
                        nc.tensor.matmul(
                            out=ps,
                            lhsT=w_sb[:, k, dt_i * 128 : (dt_i + 1) * 128],
                            rhs=xc[:, k, :],
                            start=(k == 0),
                            stop=(k == nkt - 1),
                        )
                    if dst is None:
                        nc.vector.tensor_copy(
                            out=qt_e[0:64, dt_i, ss], in_=ps[0:64, :]
                        )
                        nc.vector.tensor_copy(
                            out=qt_o[64:128, dt_i, ss], in_=ps[64:128, :]
                        )
                    else:
                        nc.vector.tensor_copy(out=dst[:, dt_i, ss], in_=ps)
                    yield
                for ssub in range(4):
                    ps = mmp.tile([128, 512], F32, tag="mm")
                    for k in range(nkt):
                        nc.tensor.matmul(
                            out=ps,
                            lhsT=xc[:, k, ssub * 128 : (ssub + 1) * 128],
                            rhs=wv_sb[:, k, :],
                            start=(k == 0),
                            stop=(k == nkt - 1),
                        )
                    kti = st * 4 + ssub
                    ps4 = ps.rearrange("p (h d) -> p h d", d=DK)
                    vv = v4[:, kti, :, :]
                    nc.vector.tensor_copy(out=vv[:, :, 0:DK], in_=ps4)
                    yield

            def attn(pr, c, filler):
                """Attention for head pair pr, q chunk c. After each kti's
                scores+exp, advances the filler iterator once to slot an
                independent matmul group into the PE queue (covers the
                exp-paced stretches)."""
                a_tiles = {}
                opses = {}
                nkts = 4 * c + 4

                def av_mms(k2, stop):
                    qoff2 = 128 * (k2 % 4) if k2 // 4 == c else 0
                    for hh in (1, 0):
                        if k2 == 0:
                            opses[hh] = otpp.tile([128, 512], F32, tag="otp", name=f"ops{pr}{c}{hh}")
                        nc.tensor.matmul(
                            out=opses[hh][0:65, qoff2:512],
                            lhsT=v[:, k2, (pr * 2 + hh) * 65 : (pr * 2 + hh) * 65 + 65],
                            rhs=a_tiles[k2][:, hh, qoff2:512],
                            start=(k2 == 0),
                            stop=stop,
                        )

                for kti in range(nkts):
                    # software-pipelined AV, lag 3, emitted before the scores
                    # so it fills the scores' PSUM-recycle wait
                    if kti >= 3:
                        av_mms(kti - 3, stop=False)
                    diag = kti // 4 == c
                    qoff = 128 * (kti % 4) if diag else 0
                    ps = scp.tile([128, 2, 512], F32, tag="sc")
                    for hh, qsrc in ((0, qt_e), (1, qt_o)):
                        nc.tensor.matmul(
                            out=ps[:, hh, qoff:512],
                            lhsT=kt_[:, pr, kti * 128 : (kti + 1) * 128],
                            rhs=qsrc[:, pr, c * 512 + qoff : (c + 1) * 512],
                            start=True,
                            stop=True,
                        )
                    at = apool.tile([128, 2, 512], BF16, tag="a")
                    nc.scalar.activation(
                        out=at[:, :, qoff:512],
                        in_=ps[:, :, qoff:512],
                        func=EXP,
                        scale=0.125,
                    )
                    if diag:
                        dg = at[:, :, qoff : qoff + 128]
                        nc.vector.tensor_tensor(
                            out=dg, in0=dg, in1=_pbcast2(mask_sb), op=MUL
                        )
                    a_tiles[kti] = at
                    next(filler, None)
                for k2 in range(max(0, nkts - 3), nkts):
                    av_mms(k2, stop=(k2 == nkts - 1))
                csl = slice(c * 512, (c + 1) * 512)
                # phase A for both heads first (copies, reciprocals, DMA
                # issues), then the multiplies: keeps hh0's copy/recip from
                # queuing behind hh1's broadcast-waiting multiply on the
                # in-order DVE queue
                rss, bcs = {}, {}
                for hh in (1, 0):
                    ops = opses[hh]
                    rs = rcpp.tile([128, 512], F32, tag="rs")
                    nc.vector.tensor_copy(out=rs[0:65, :], in_=ops[0:65, :])
                    rr = rcpp.tile([65, 512], F32, tag="rr")
                    nc.vector.reciprocal(out=rr[64:65, :], in_=rs[64:65, :])
                    rd = rbp.tile([1, 512], F32, tag="rb")
                    nc.sync.dma_start(out=rd, in_=rr[64:65, :])
                    bc = bcp.tile([128, 512], F32, tag="bc")
                    nc.sync.dma_start(out=bc[0:64, :], in_=_pbcast(rd, 64))
                    rss[hh], bcs[hh] = rs, bc
                for hh in (1, 0):  # hh=0 last: ends with a DVE write, no DMA
                    rs, bc = rss[hh], bcs[hh]
                    if hh == 0:
                        nc.vector.tensor_tensor(
                            out=ot[0:64, pr, csl],
                            in0=rs[0:64, :],
                            in1=bc[0:64, :],
                            op=MUL,
                        )
                    else:
                        tmp = bcp.tile([64, 512], BF16, tag="tmp")
                        nc.vector.tensor_tensor(
                            out=tmp, in0=rs[0:64, :], in1=bc[0:64, :], op=MUL
                        )
                        nc.sync.dma_start(out=ot[64:128, pr, csl], in_=tmp)

            def proj_steps(chunk, last_pair=None):
                """Generator: one yield per srow group. When last_pair is
                given, that pair's matmul goes last in each accumulation so
                the other three overlap its still-running softmax dance."""
                dts = list(range(NPAIR))
                if last_pair is not None:
                    dts.remove(last_pair)
                    dts.append(last_pair)
                for ssub in range(4):
                    srow = chunk * 4 + ssub
                    ob = obp.tile([128, 2, 512], F32, tag="ob")
                    for mc in range(2):
                        ps = mmp.tile([128, 512], F32, tag="mm")
                        for j, dt_i in enumerate(dts):
                            nc.tensor.matmul(
                                out=ps,
                                lhsT=ot[:, dt_i, srow * 128 : (srow + 1) * 128],
                                rhs=wo_sb[:, dt_i, mc * 512 : (mc + 1) * 512],
                                start=(j == 0),
                                stop=(j == NPAIR - 1),
                            )
                        nc.vector.tensor_copy(out=ob[:, mc, :], in_=ps)
                        nc.sync.dma_start(
                            out=out[
                                srow * 128 : (srow + 1) * 128,
                                mc * 512 : (mc + 1) * 512,
                            ],
                            in_=ob[:, mc, :],
                        )
                    yield

            def proj(chunk, last_pair=None):
                for _ in proj_steps(chunk, last_pair):
                    pass

            pp_tiles = {}

            def proj_last_p1(last_pair):
                """First 3 pairs of the final chunk's projection, closed
                groups copied to bf16 partials so all of it can run during
                the final pair's attention."""
                chunk = SC - 1
                dts = [d for d in range(NPAIR) if d != last_pair]
                for ssub in range(4):
                    srow = chunk * 4 + ssub
                    for mc in range(2):
                        ps = mmp.tile([128, 512], F32, tag="mm")
                        for j, dt_i in enumerate(dts):
                            nc.tensor.matmul(
                                out=ps,
                                lhsT=ot[:, dt_i, srow * 128 : (srow + 1) * 128],
                                rhs=wo_sb[:, dt_i, mc * 512 : (mc + 1) * 512],
                                start=(j == 0),
                                stop=(j == len(dts) - 1),
                            )
                        pp = ppp.tile(
                            [128, 512], BF16, tag="pp", name=f"pp{ssub}{mc}"
                        )
                        nc.vector.tensor_copy(out=pp, in_=ps)
                        pp_tiles[(ssub, mc)] = pp
                        yield

            def proj_last_p2(last_pair):
                chunk = SC - 1
                for ssub in range(4):
                    srow = chunk * 4 + ssub
                    ob = obp.tile([128, 2, 512], F32, tag="ob")
                    for mc in range(2):
                        ps = mmp.tile([128, 512], F32, tag="mm")
                        nc.tensor.matmul(
                            out=ps,
                            lhsT=ot[:, last_pair, srow * 128 : (srow + 1) * 128],
                            rhs=wo_sb[:, last_pair, mc * 512 : (mc + 1) * 512],
                            start=True,
                            stop=True,
                        )
                        t2 = bcp.tile([128, 512], F32, tag="bc")
                        nc.scalar.copy(out=t2, in_=ps)
                        nc.vector.tensor_tensor(
                            out=ob[:, mc, :],
                            in0=t2,
                            in1=pp_tiles[(ssub, mc)],
                            op=mybir.AluOpType.add,
                        )
                        nc.sync.dma_start(
                            out=out[
                                srow * 128 : (srow + 1) * 128,
                                mc * 512 : (mc + 1) * 512,
                            ],
                            in_=ob[:, mc, :],
                        )

            def _seq(*gens):
                for g in gens:
                    yield from g

            def _skips(n):
                for _ in range(n):
                    yield

            empty = iter(())
            for st in range(SC):
                for _ in qkv_steps(st, xc=xc0 if st == 0 else None):
                    pass
                last = st == SC - 1
                if st >= 1 and not last:
                    proj(st - 1)
                pairs = (3, 0, 1, 2) if last else (0, 1, 2, 3)
                # the last stile has no next-stile qkv to fill the exp-paced
                # stretches, so feed proj(SC-2) groups in as filler instead
                filler = proj_steps(SC - 2) if last else empty
                for pr in pairs:
                    attn(pr, st, filler)
                for _ in filler:
                    pass
            # final projection in two passes: the three finished pairs run
            # inside the last softmax-dance latency (keeps PE busy and warm),
            # the last pair joins via a single matmul + SBUF add afterwards
            for _ in proj_last_p1(pairs[-1]):
                pass
            # keep the PE clock warm across the final dance latency so the
            # last-pair matmuls run at full rate
            wps2 = mmp.tile([128, 512], F32, tag="mm")
            for i in range(10):
                nc.tensor.matmul(
                    out=wps2,
                    lhsT=warm[:, 0, 0:128],
                    rhs=warm[:, 1, :],
                    start=(i == 0),
                    stop=(i == 9),
                )
            proj_last_p2(pairs[-1])
    return nc


_NC_CACHE = {}


def _get_nc(nkt):
    if nkt not in _NC_CACHE:
        nc = bacc.Bacc(
            "TRN2",
            target_bir_lowering=False,
            debug=False,
            enable_asserts=False,
            num_devices=8,
        )
        _emit(nc, nkt)
        nc.compile()
        _NC_CACHE[nkt] = nc
    return _NC_CACHE[nkt]


def kernel(**inputs):
    x = np.asarray(inputs["x"], dtype=np.float32)
    mask = np.asarray(inputs["mask"]).reshape(S, S)
    Wq = np.asarray(inputs["Wq"], dtype=np.float32)
    bq = np.asarray(inputs["bq"], dtype=np.float32)
    Wk = np.asarray(inputs["Wk"], dtype=np.float32)
    bk = np.asarray(inputs["bk"], dtype=np.float32)
    Wv = np.asarray(inputs["Wv"], dtype=np.float32)
    bv = np.asarray(inputs["bv"], dtype=np.float32)
    Wo = np.asarray(inputs["Wo"], dtype=np.float32)
    bo = np.asarray(inputs["bo"], dtype=np.float32)

    assert np.array_equal(
        mask, np.tril(np.ones((S, S), dtype=bool))
    ), "kernel specialized for the causal (tril) mask"

    bias_zero = not (bq.any() or bk.any() or bv.any())
    nkt = 8 if bias_zero else 9
    nc = _get_nc(nkt)

    # local diag-block mask in (k, q) layout: valid when q >= k
    mtile = np.triu(np.ones((128, 128), dtype=np.float32)).astype(BF)

    def aug(w, b):
        if bias_zero:
            return w.astype(BF)
        pad = np.zeros((128, w.shape[1]), dtype=np.float32)
        pad[0] = b
        return np.vstack([w, pad]).astype(BF)

    in_maps = []
    for core in range(8):
        b, hg = divmod(core, 2)
        cols = slice(hg * DQK, (hg + 1) * DQK)
        xT = x[b].T
        if not bias_zero:
            pad = np.zeros((128, S), dtype=np.float32)
            pad[0] = 1.0
            xT = np.vstack([xT, pad])
        in_maps.append(
            {
                "xT": np.ascontiguousarray(xT).astype(BF),
                "wq": aug(Wq[:, cols], bq[cols]),
                "wk": aug(Wk[:, cols], bk[cols]),
                "wv": aug(Wv[:, cols], bv[cols]),
                "wo": np.ascontiguousarray(Wo[cols, :]).astype(BF),
                "mask": mtile,
            }
        )

    res = run_bass_kernel_spmd(nc, in_maps, core_ids=list(range(8)))
    global LAST_RESULT
    LAST_RESULT = res
    parts = [r["out"] for r in res.results]
    out = np.stack(
        [parts[2 * b_] + parts[2 * b_ + 1] for b_ in range(B)]
    ) + bo.astype(np.float32)
    return out.astype(np.float32)
